# revision 41
# baseline (speedup 1.0000x reference)
"""DirectedDualPNA on 8 Trainium2 NeuronCores — v3.

Strategy (node-sharded):
  m_e = h_e @ pre_W + pre_b with h_e = [x_dst | x_src] decomposes as
  m_e = A[dst] + B[src],  A = x @ pre_W[:F] + pre_b,  B = x @ pre_W[F:].
  Per-dst segment stats of m reduce to segment stats of B[src]:
    sum   = cnt*A + sum(B);   mean = sum/safe
    var   = E[B^2] - E[B]^2   (A shifts cancel)
    min   = A + min(B); max = A + max(B)   (0 when cnt==0)

v3 performance structure (~6.2-6.4ms vs v2's 7.4-8.2ms):
  - Equal-K chunk schedule with k-OUTER gather layout: all groups in a chunk
    share one slot count and idx flat order is (k, g, p), so each chunk
    issues ONE idx load + TWO dma_gathers (lo/hi windows; merged calls get
    Q7 descriptor-gen to ~2.6ns/desc vs ~8.6ns for per-group calls) and the
    gathered tile is [p, k, (g f)] with each k-slab contiguous - segment
    stats are chunk-wide contiguous halving-fold trees at full DVE rate
    (strided [p,g,f,k] tensor_reduce measured ~3x slower; 16-bit tables
    fail the 2e-2 max-rel tolerance, so everything stays fp32).
  - sum and sumsq fold lo+hi in one tree each; sumsq via in-place ACT
    Square on the gathered tile. min/max corrections (and the nonempty
    mask) are skipped entirely for chunks with no empty lo/hi halves.
  - Directions interleaved inside the chunk loop (independent dep chains
    keep Pool descriptor-gen and DVE folds overlapped).
  - lin (64x64) folded into ppw/acatw/biases on the host (per-node scale
    factors commute with right-multiplication), killing the per-group
    transpose + 2 matmuls; relu is one chunk-wide ACT op.
  - Post-matmul transposes batched into one PSUM tile (2 copies not 5).
  - Layer-1 B-table build fused into layer-0 phase D (x2 already in SBUF,
    stored both row-major for the x2 perm-gather and transposed for
    layer-1 phase D / AX lhsT loads).
  - variance clamp via TT max against a zeros broadcast (tensor_scalar_max
    measured ~19us/op pathological).
"""

import os
import numpy as np

import concourse.bass as bass
import concourse.mybir as mybir
from concourse import bacc
from concourse.bass_utils import run_bass_kernel_spmd
from concourse.tile import TileContext
from concourse.masks import make_identity

P = 128
F = 128
NCORES = 8
LAYERS = 2
EPS = 1e-5
BIG = 1e30
G = 3                       # max groups per chunk
NQ = 4                      # swdge queues
FP32 = mybir.dt.float32
BF16 = mybir.dt.bfloat16
I16 = mybir.dt.int16


# ----------------------------------------------------------------- host prep

def _wrap16(flat):
    """[n] int16 -> wrapped [128, n//16]: position j lives at (j%16, j//16),
    replicated across the 8 Q7 cores (every 16 partitions)."""
    n = flat.shape[0]
    assert n % 16 == 0
    w = flat.reshape(n // 16, 16).T.astype(np.int16)
    return np.tile(w, (8, 1))


def _chunk_list(ng):
    """Small chunks first (high-degree groups), then G-sized chunks."""
    out = []
    rem = ng
    for h in (1, 2, 2):
        if rem >= h:
            out.append(h)
            rem -= h
    while rem > 0:
        t = min(G, rem)
        out.append(t)
        rem -= t
    return out


def _prep_direction(edge_index, n_nodes, nown, low_w, hib, avg_log):
    """Host-side per-direction prep. Returns per-core dict + shared K sched.

    Overlapping index windows: lo = table rows [0, low_w), hi = [hib, tbl).
    Edges with src in the overlap [hib, low_w) are assigned to whichever
    side balances the per-node lo/hi slot counts. All groups in a chunk
    share one (K_lo, K_hi) slot count (max over groups and cores)."""
    src = np.asarray(edge_index[0], dtype=np.int64)
    dst = np.asarray(edge_index[1], dtype=np.int64)
    ng = (nown + P - 1) // P
    nownp = ng * P
    chunks = _chunk_list(ng)
    nchunks = len(chunks)
    g0s = np.concatenate([[0], np.cumsum(chunks)])[:-1]
    cores = []
    for c in range(NCORES):
        sel = (dst >= c * nown) & (dst < (c + 1) * nown)
        es = src[sel]
        ed = dst[sel] - c * nown
        cnt = np.bincount(ed, minlength=nown)
        order = np.argsort(ed, kind="stable")
        es_sorted = es[order]
        starts = np.zeros(nown + 1, np.int64)
        np.cumsum(cnt, out=starts[1:])
        perm = np.argsort(-cnt, kind="stable")
        lo_lists = []
        hi_lists = []
        for j in range(nownp):
            if j < nown:
                n = perm[j]
                s = es_sorted[starts[n]:starts[n + 1]]
                lo_only = s[s < hib]
                hi_only = s[s >= low_w]
                flex = s[(s >= hib) & (s < low_w)]
                a = min(max((len(hi_only) + len(flex) - len(lo_only) + 1) // 2, 0),
                        len(flex))
                lo_lists.append(np.concatenate([lo_only, flex[:a]]))
                hi_lists.append(np.concatenate([flex[a:], hi_only]))
            else:
                lo_lists.append(np.empty(0, np.int64))
                hi_lists.append(np.empty(0, np.int64))
        clo = np.array([len(v) for v in lo_lists], np.int64)
        chi = np.array([len(v) for v in hi_lists], np.int64)
        cores.append(dict(cnt=cnt, perm=perm, lo=lo_lists, hi=hi_lists,
                          clo=clo, chi=chi))
    # shared per-chunk slot schedule
    K_lo = np.zeros(nchunks, np.int64)
    K_hi = np.zeros(nchunks, np.int64)
    for ci in range(nchunks):
        s, e = g0s[ci] * P, (g0s[ci] + chunks[ci]) * P
        K_lo[ci] = max(1, max(int(cc["clo"][s:e].max()) for cc in cores))
        K_hi[ci] = max(1, max(int(cc["chi"][s:e].max()) for cc in cores))
    dummy_hi = n_nodes - hib  # a zeroed table row, in hi-window coords
    out_cores = []
    for c in range(NCORES):
        cc = cores[c]
        idx_parts = []
        scal = np.zeros((nchunks, P, G * 16), np.float32)
        for ci in range(nchunks):
            Gc = chunks[ci]
            kl, kh = int(K_lo[ci]), int(K_hi[ci])
            # k-outer packing: flat idx order (k, g, p) so the gathered tile
            # is [p, k, g, f] with each k-slab [Gc*F] contiguous
            slo = np.zeros((kl, Gc, P), np.int64)
            shi = np.full((kh, Gc, P), dummy_hi, np.int64)
            for gj in range(Gc):
                g = g0s[ci] + gj
                for p in range(P):
                    j = g * P + p
                    lo, hi = cc["lo"][j], cc["hi"][j]
                    nl_, nh_ = len(lo), len(hi)
                    if nl_ > 0:
                        slo[:nl_, gj, p] = lo
                        slo[nl_:, gj, p] = lo[0]
                    # else stays 0 (row 0; dup + sum-corrected)
                    if nh_ > 0:
                        shi[:nh_, gj, p] = hi - hib
                        shi[nh_:, gj, p] = hi[0] - hib
                    cntj = cc["cnt"][cc["perm"][j]] if j < nown else 0
                    safe = max(cntj, 1)
                    logd = np.log(safe + 1.0)
                    o = gj * 16
                    scal[ci, p, o + 0] = -(kl - nl_)          # npadlo
                    scal[ci, p, o + 1] = -(kh - nh_)          # npadhi
                    scal[ci, p, o + 2] = 0.0 if nl_ > 0 else BIG   # mlo_big
                    scal[ci, p, o + 3] = 0.0 if nh_ > 0 else BIG   # mhi_big
                    scal[ci, p, o + 4] = 1.0 / safe           # recip
                    scal[ci, p, o + 5] = 1.0 if cntj > 0 else 0.0  # nonempty
                    scal[ci, p, o + 6] = logd / avg_log       # f1
                    scal[ci, p, o + 7] = avg_log / logd       # f2
                    scal[ci, p, o + 8] = float(cntj)          # cnt
            idx_parts.append(_wrap16(slo.reshape(-1).astype(np.int16)))
            idx_parts.append(_wrap16(shi.reshape(-1).astype(np.int16)))
        perm_pad = np.concatenate([cc["perm"], np.zeros(nownp - nown, np.int64)])
        # x2 perm gather: perm position j -> local row perm_pad[j] (pad -> nown)
        x2src = perm_pad.copy()
        x2src[nown:] = nown  # zeroed pad row
        x2pidx = _wrap16(x2src.astype(np.int16))
        # h scatter: perm position j -> natural row perm_pad[j] (pad -> dummy)
        hdest = perm_pad.copy()
        hdest[nown:] = nownp  # dummy row
        h_idx = _wrap16(hdest.astype(np.int16))
        out_cores.append(dict(
            idx_cat=np.concatenate(idx_parts, axis=1),
            scal=scal, perm_pad=perm_pad, x2pidx=x2pidx, h_idx=h_idx,
        ))
    # per-chunk "clean" flag: every node (incl pads) has both halves nonempty
    clean = np.ones(nchunks, bool)
    for cc in cores:
        for ci in range(nchunks):
            s, e = g0s[ci] * P, (g0s[ci] + chunks[ci]) * P
            if (cc["clo"][s:e].min() == 0) or (cc["chi"][s:e].min() == 0):
                clean[ci] = False
    return out_cores, K_lo, K_hi, ng, nownp, chunks, g0s, clean


def _prep_weights(inputs, l):
    """Per-layer/direction packed weights (numpy)."""
    w = {}
    for d, tag in enumerate(("in", "out")):
        preW = np.asarray(inputs[f"{tag}_pre_W"][l], np.float32)    # [256,128]
        preB = np.asarray(inputs[f"{tag}_pre_b"][l], np.float32)    # [128]
        postW = np.asarray(inputs[f"{tag}_post_W"][l], np.float32)  # [2048,64]
        postB = np.asarray(inputs[f"{tag}_post_b"][l], np.float32)  # [64]
        linW = np.asarray(inputs[f"{tag}_lin_W"][l], np.float32)    # [64,64]
        linB = np.asarray(inputs[f"{tag}_lin_b"][l], np.float32)    # [64]
        w[(d, "wbot")] = preW[F:2 * F]                               # [128,128]
        # linW/linB folded into the post weights: per-node scale factors
        # commute with the right-multiplication by linW.
        w[(d, "acatw")] = np.concatenate([preW[0:F], postW[0:F] @ linW], axis=1)
        w[(d, "acatb")] = np.tile(np.concatenate([preB, postB @ linW + linB])[None, :], (P, 1))
        pp = np.zeros((F, 5 * 192), np.float32)
        for p_ in range(5):
            for k in range(3):
                rows = postW[F + k * 5 * F + p_ * F: F + k * 5 * F + (p_ + 1) * F]
                pp[:, p_ * 192 + k * 64: p_ * 192 + (k + 1) * 64] = rows @ linW
        w[(d, "ppw")] = pp
    w["combw"] = np.asarray(inputs["comb_W"][l], np.float32)         # [256,128]
    w["combb"] = np.tile(np.asarray(inputs["comb_b"][l], np.float32)[None, :], (P, 1))
    return w


def _seg_fold(scratch, src_tile, off, K, op, out_ap, v, W):
    """out[128,W] = reduce(src_tile[:, off : off+K*W] viewed [K,W], op) along K.
    Contiguous halving fold (odd blocks deferred)."""
    tt = v.tensor_tensor
    if K == 1:
        v.tensor_copy(out_ap, src_tile[:, off:off + W])
        return
    if K == 2:
        tt(out_ap, src_tile[:, off:off + W], src_tile[:, off + W:off + 2 * W], op=op)
        return
    if K == 3:
        tt(scratch[:, 0:W], src_tile[:, off:off + W],
           src_tile[:, off + W:off + 2 * W], op=op)
        tt(out_ap, scratch[:, 0:W], src_tile[:, off + 2 * W:off + 3 * W], op=op)
        return
    pend = []
    h = K // 2
    tt(scratch[:, 0:h * W], src_tile[:, off:off + h * W],
       src_tile[:, off + h * W:off + 2 * h * W], op=op)
    if K - 2 * h:
        pend.append((src_tile, off + 2 * h * W))
    cur = h
    while cur > 2:
        h2 = cur // 2
        tt(scratch[:, 0:h2 * W], scratch[:, 0:h2 * W],
           scratch[:, h2 * W:2 * h2 * W], op=op)
        if cur - 2 * h2:
            pend.append((scratch, 2 * h2 * W))
        cur = h2
    if cur == 2:
        operands = [(scratch, 0), (scratch, W)] + pend
    else:
        operands = [(scratch, 0)] + pend
    n = len(operands)
    at, ao = operands[0]
    for i in range(1, n):
        bt, bo = operands[i]
        dst = out_ap if i == n - 1 else scratch[:, 0:W]
        tt(dst, at[:, ao:ao + W], bt[:, bo:bo + W], op=op)
        at, ao = scratch, 0
    return


# -------------------------------------------------------------- device build

def build_program(meta):
    """Build the SPMD bass program (shared by all 8 cores)."""
    n_nodes = meta["n_nodes"]
    low_w = meta["low_w"]
    hib = meta["hib"]
    tbl = meta["tbl"]
    nown = meta["nown"]
    nownp = meta["nownp"]
    ng = meta["ng"]
    chunks = meta["chunks"]
    g0s = meta["g0s"]
    K_lo = meta["K_lo"]    # dict d -> [nchunks]
    K_hi = meta["K_hi"]
    clean = meta["clean"]
    nchunks = len(chunks)
    # widest chunk (slots) for tile padding
    maxw = max(int(chunks[ci] * (K_lo[d][ci] + K_hi[d][ci]))
               for d in (0, 1) for ci in range(nchunks))
    sum_cols8 = {d: sum(int(chunks[ci]) * int(K_lo[d][ci] + K_hi[d][ci]) * 8
                        for ci in range(nchunks)) for d in (0, 1)}
    HT = nownp + P                  # h table rows (incl dummy)

    nc = bacc.Bacc("TRN2", target_bir_lowering=False, debug=False,
                   num_devices=NCORES, num_swdge_queues=NQ)

    # ---- DRAM I/O
    xTown = nc.dram_tensor("xTown", [P, nownp], FP32, kind="ExternalInput")
    xTownP = {d: nc.dram_tensor(f"xTownP{d}", [P, nownp], FP32, kind="ExternalInput") for d in (0, 1)}
    idx_cat = {d: nc.dram_tensor(f"idxcat{d}", [P, sum_cols8[d]], I16, kind="ExternalInput") for d in (0, 1)}
    scal_t = {d: nc.dram_tensor(f"scal{d}", [nchunks, P, G * 16], FP32, kind="ExternalInput") for d in (0, 1)}
    x2pidx_t = {d: nc.dram_tensor(f"x2pidx{d}", [P, nownp // 16], I16, kind="ExternalInput") for d in (0, 1)}
    h_idx_t = {d: nc.dram_tensor(f"hidx{d}", [P, nownp // 16], I16, kind="ExternalInput") for d in (0, 1)}
    wbot_t = nc.dram_tensor("wbot", [LAYERS, 2, F, F], FP32, kind="ExternalInput")
    acatw_t = nc.dram_tensor("acatw", [LAYERS, 2, F, 192], FP32, kind="ExternalInput")
    acatb_t = nc.dram_tensor("acatb", [LAYERS, 2, P, 192], FP32, kind="ExternalInput")
    ppw_t = nc.dram_tensor("ppw", [LAYERS, 2, F, 5 * 192], FP32, kind="ExternalInput")
    combw_t = nc.dram_tensor("combw", [LAYERS, 256, F], FP32, kind="ExternalInput")
    combb_t = nc.dram_tensor("combb", [LAYERS, P, F], FP32, kind="ExternalInput")
    headw_t = nc.dram_tensor("headw", [F, 8], FP32, kind="ExternalInput")
    headb_t = nc.dram_tensor("headb", [P, 8], FP32, kind="ExternalInput")
    out_t = nc.dram_tensor("out", [nown, 8], FP32, kind="ExternalOutput")
    dbg = bool(int(os.environ.get("PNA_DEBUG_DUMP", "0")))
    if dbg:
        st_dump = nc.dram_tensor("st_dump", [nchunks, P, 8 * G * F], FP32, kind="ExternalOutput")
        so_dump = nc.dram_tensor("so_dump", [nchunks, P, 5 * G * F], FP32, kind="ExternalOutput")
        gt_dump = nc.dram_tensor("gt_dump", [nchunks, P, maxw * F], FP32, kind="ExternalOutput")
        pys_dump = nc.dram_tensor("pys_dump", [nchunks, P, G * 192], FP32, kind="ExternalOutput")
        hsb_dump = nc.dram_tensor("hsb_dump", [nchunks, P, G * 64], FP32, kind="ExternalOutput")
        htab_dump = {dd: nc.dram_tensor(f"htab_dump{dd}", [HT, 64], FP32, kind="ExternalOutput") for dd in (0, 1)}

    # ---- DRAM internal
    Bt = {d: nc.dram_tensor(f"Bt{d}", [tbl, F], FP32, addr_space="Shared") for d in (0, 1)}
    Btown = {d: nc.dram_tensor(f"Btown{d}", [nown, F], FP32) for d in (0, 1)}
    h_tab_l = {(l, d): nc.dram_tensor(f"htab{l}{d}", [HT, 64], FP32)
               for l in range(LAYERS) for d in (0, 1)}
    x2own = nc.dram_tensor("x2own", [nownp, F], FP32)
    x2ownT = nc.dram_tensor("x2ownT", [P, nownp], FP32)

    AF = mybir.ActivationFunctionType
    OP = mybir.AluOpType
    AX = mybir.AxisListType

    with TileContext(nc) as tc:
        with tc.tile_pool(name="const", bufs=1) as constp, \
             tc.tile_pool(name="wts", bufs=1) as wtsp, \
             tc.tile_pool(name="xt", bufs=2) as xtp, \
             tc.tile_pool(name="ip", bufs=3) as ipool, \
             tc.tile_pool(name="gath", bufs=2) as gathp, \
             tc.tile_pool(name="chk", bufs=1) as chkp, \
             tc.tile_pool(name="s0p", bufs=2) as s0p, \
             tc.tile_pool(name="axp", bufs=2) as axp, \
             tc.tile_pool(name="pys", bufs=2) as pysp, \
             tc.tile_pool(name="nl", bufs=1) as nlp, \
             tc.tile_pool(name="ps", bufs=2, space="PSUM") as psp, \
             tc.tile_pool(name="psa", bufs=2, space="PSUM") as psap:

            ident = constp.tile([P, P], FP32)
            make_identity(nc, ident[:])
            zeros = constp.tile([P, 128], FP32)
            nc.vector.memset(zeros[:], 0.0)
            eps_col = constp.tile([P, 1], FP32)
            nc.vector.memset(eps_col[:], EPS)
            ones1 = constp.tile([1, P], FP32)
            nc.vector.memset(ones1[:], 1.0)

            def load_w(pool, dram_ap, shape, tag):
                t = pool.tile(shape, FP32, tag=tag, name=tag)
                nc.sync.dma_start(out=t[:], in_=dram_ap)
                return t

            headw_sb = load_w(constp, headw_t[:], [F, 8], "headw")
            headb_sb = load_w(constp, headb_t[:], [P, 8], "headb")
            wbot_sb = {(l, d): load_w(constp, wbot_t[l, d], [F, F], f"wbot{l}{d}")
                       for l in range(LAYERS) for d in (0, 1)}

            def zero_dram_rows(tensor, r0, r1, width, ztile):
                if r1 <= r0:
                    return
                flat = tensor[r0:r1, :].rearrange("n f -> (n f)")
                total = (r1 - r0) * width
                assert total % P == 0
                per = total // P
                vv = flat.rearrange("(p x) -> p x", p=P)
                off = 0
                while off < per:
                    w = min(128, per - off)
                    nc.sync.dma_start(out=vv[:, off:off + w], in_=ztile[:, 0:w])
                    off += w

            def transpose_to_sbuf(src_ap, tag):
                """PE transpose src [128, <=128] -> sbuf (scalar copy out)."""
                pt = psp.tile([P, P], FP32, tag="ptr")
                nc.tensor.transpose(out=pt[:], in_=src_ap, identity=ident[:])
                sb = xtp.tile([P, P], FP32, tag=tag, name=tag)
                nc.scalar.copy(sb[:], pt[:])
                return sb

            # one-time zeroing
            for d in (0, 1):
                zero_dram_rows(Bt[d], n_nodes, tbl, F, zeros)
                for l in range(LAYERS):
                    zero_dram_rows(h_tab_l[(l, d)], 0, HT, 64, zeros)
            zero_dram_rows(x2own, nown, nownp, F, zeros)

            for l in range(LAYERS):
                combw1_sb = load_w(wtsp, combw_t[l, 0:F, :], [F, F], "combw1")
                combw2_sb = load_w(wtsp, combw_t[l, F:256, :], [F, F], "combw2")
                combb_sb = load_w(wtsp, combb_t[l], [P, F], "combb")

                h_tab = {d: h_tab_l[(l, d)] for d in (0, 1)}

                # ---------- phase A: sharded B-table build + AllGather
                # (layer 1's Btown is produced during layer-0 phase D)
                if l == 0:
                    for t in range(ng):
                        if t % G == 0:
                            nb = min(G, ng - t)
                            xbb = xtp.tile([P, nb * P], FP32, tag="xpc",
                                           padded_shape=[P, G * P], name="xpc")
                            nc.sync.dma_start(out=xbb[:], in_=xTown[:, t * P:(t + nb) * P])
                        lhsT_ap = xbb[:, (t % G) * P:((t % G) + 1) * P]
                        rows_o = min(P, nown - t * P)
                        for d in (0, 1):
                            pb = psp.tile([P, P], FP32, tag="pb")
                            nc.tensor.matmul(out=pb[:], lhsT=lhsT_ap, rhs=wbot_sb[(0, d)][:],
                                             start=True, stop=True)
                            sb = xtp.tile([P, P], FP32, tag="bsb", name="bsb")
                            nc.scalar.copy(sb[:], pb[:])
                            nc.sync.dma_start(out=Btown[d][t * P:t * P + rows_o, :],
                                              in_=sb[:rows_o, :])
                for d in (0, 1):
                    nc.gpsimd.collective_compute(
                        "AllGather", OP.bypass,
                        replica_groups=[list(range(NCORES))],
                        ins=[Btown[d][:]], outs=[Bt[d][0:n_nodes, :]])

                # per-direction weights + idx tables (both directions resident)
                acatw_sb = {}
                acatb_sb = {}
                ppw_sb = {}
                hix = {}
                x2pix = {}
                for d in (0, 1):
                    acatw_sb[d] = load_w(wtsp, acatw_t[l, d], [F, 192], f"acatw{d}")
                    acatb_sb[d] = load_w(wtsp, acatb_t[l, d], [P, 192], f"acatb{d}")
                    ppw_sb[d] = load_w(wtsp, ppw_t[l, d], [F, 5 * 192], f"ppw{d}")
                    hix[d] = wtsp.tile([P, nownp // 16], I16, tag=f"hix{d}", name="hix")
                    nc.sync.dma_start(out=hix[d][:], in_=h_idx_t[d][:])
                    if l == 1:
                        x2pix[d] = wtsp.tile([P, nownp // 16], I16, tag=f"x2pix{d}", name="x2pix")
                        nc.sync.dma_start(out=x2pix[d][:], in_=x2pidx_t[d][:])

                off8 = {0: 0, 1: 0}
                hsb_prev = None  # deferred chunk scatter
                v = nc.vector

                def scatter_chunk(entry):
                    hs, dd, sg0, gcc = entry
                    nc.gpsimd.dma_scatter_add(
                        out_ap=h_tab[dd][:],
                        in_ap=hs[:, 0:gcc * 64].rearrange("p (c w) -> p c w", w=64),
                        idxs_ap=hix[dd][:, sg0 * 8:(sg0 + gcc) * 8],
                        num_idxs=gcc * P, num_idxs_reg=gcc * P,
                        elem_size=64, single_packet=False, queue_num=0)

                for ci in range(nchunks):
                    Gc = chunks[ci]
                    g0 = int(g0s[ci])
                    for d in (0, 1):
                        KLc = int(K_lo[d][ci])
                        KHc = int(K_hi[d][ci])
                        nlo = Gc * KLc        # lo slots (cols of F)
                        nhi = Gc * KHc
                        w8 = (nlo + nhi) * 8
                        il = ipool.tile([P, w8], I16, tag="il",
                                        padded_shape=[P, maxw * 8], bufs=2, name="il")
                        nc.sync.dma_start(out=il[:], in_=idx_cat[d][:, off8[d]:off8[d] + w8])
                        off8[d] += w8

                        if l == 1:
                            x2g = ipool.tile([P, Gc * F], FP32, tag="x2g",
                                             padded_shape=[P, G * F], bufs=2, name="x2g")
                            nc.gpsimd.dma_gather(
                                out_ap=x2g[:].rearrange("p (c w) -> p c w", w=F),
                                in_ap=x2own[:], idxs_ap=x2pix[d][:, g0 * 8:(g0 + Gc) * 8],
                                num_idxs=Gc * P, num_idxs_reg=Gc * P,
                                elem_size=F, single_packet=False, queue_num=0)

                        gt = gathp.tile([P, (nlo + nhi) * F], FP32, tag="gt",
                                        padded_shape=[P, maxw * F], name="gt")
                        nc.gpsimd.dma_gather(
                            out_ap=gt[:, 0:nlo * F].rearrange("p (c w) -> p c w", w=F),
                            in_ap=Bt[d][0:low_w, :], idxs_ap=il[:, 0:nlo * 8],
                            num_idxs=nlo * P, num_idxs_reg=nlo * P,
                            elem_size=F, single_packet=False, queue_num=0)
                        nc.gpsimd.dma_gather(
                            out_ap=gt[:, nlo * F:(nlo + nhi) * F].rearrange("p (c w) -> p c w", w=F),
                            in_ap=Bt[d][hib:tbl, :], idxs_ap=il[:, nlo * 8:w8],
                            num_idxs=nhi * P, num_idxs_reg=nhi * P,
                            elem_size=F, single_packet=False, queue_num=0)

                        scs = ipool.tile([P, G * 16], FP32, tag="scs", bufs=2, name="scs")
                        nc.sync.dma_start(out=scs[:], in_=scal_t[d][ci])

                        # st slots: 0=sum(all) 1=sumsq(all) 3=mnlo 4=mnhi
                        #           5=mxlo 6=mxhi  (2 unused)
                        st = chkp.tile([P, 6 * G * F], FP32, tag="st", bufs=2, name="st")
                        s0c = s0p.tile([P, 2 * G * F], FP32, tag="s0c", bufs=1, name="s0c")

                        _SLOT = {0: 0, 1: 1, 3: 2, 4: 3, 5: 4, 6: 5}

                        def slot(s):
                            s = _SLOT[s]
                            return st[:, s * G * F:(s * G * F) + Gc * F]

                        W = Gc * F          # one k-slab
                        allk = KLc + KHc

                        # stash slot-0 slabs (contiguous in k-outer layout) -
                        # must read gt BEFORE the in-place squares below
                        nc.scalar.copy(s0c[:, 0:Gc * F], gt[:, 0:W])
                        nc.scalar.copy(s0c[:, G * F:(G + Gc) * F],
                                       gt[:, nlo * F:nlo * F + W])

                        fsc = chkp.tile([P, (maxw // 2 + 2) * F], FP32,
                                        tag="fsc", name="fsc")
                        # full-rate contiguous fold trees over k slabs
                        _seg_fold(fsc, gt, 0, allk, OP.add, slot(0), v, W)
                        _seg_fold(fsc, gt, 0, KLc, OP.min, slot(3), v, W)
                        _seg_fold(fsc, gt, nlo * F, KHc, OP.min, slot(4), v, W)
                        _seg_fold(fsc, gt, 0, KLc, OP.max, slot(5), v, W)
                        _seg_fold(fsc, gt, nlo * F, KHc, OP.max, slot(6), v, W)
                        # sumsq: square gathered rows in place, then fold
                        nc.scalar.activation(gt[:, 0:(nlo + nhi) * F],
                                             gt[:, 0:(nlo + nhi) * F], AF.Square)
                        _seg_fold(fsc, gt, 0, allk, OP.add, slot(1), v, W)

                        if dbg and d == 0 and l == 0:
                            nc.sync.dma_start(out=st_dump[ci, :, :], in_=st[:, 0:8 * G * F])
                            gtf = chkp.tile([P, maxw * F], FP32, tag="gtf", name="gtf")
                            nc.vector.tensor_copy(gtf[:, 0:(nlo + nhi) * F], gt[:, 0:(nlo + nhi) * F])
                            nc.sync.dma_start(out=gt_dump[ci, :, 0:(nlo + nhi) * F], in_=gtf[:, 0:(nlo + nhi) * F])

                        # ---- A|X0 for the chunk (perm order), bias via PE
                        AXc = axp.tile([P, G * 192], FP32, tag="AXc", bufs=1, name="AXc")
                        if l == 0:
                            xpc = xtp.tile([P, Gc * P], FP32, tag="xpc",
                                           padded_shape=[P, G * P], bufs=2, name="xpc")
                            nc.sync.dma_start(out=xpc[:],
                                              in_=xTownP[d][:, g0 * P:(g0 + Gc) * P])
                        for gj in range(Gc):
                            g = g0 + gj
                            if l == 0:
                                lhsT = xpc[:, gj * P:(gj + 1) * P]
                            else:
                                lhsT = transpose_to_sbuf(x2g[:, gj * F:(gj + 1) * F], "xtt")[:]
                            pa = psap.tile([P, 192], FP32, tag="pa")
                            nc.tensor.matmul(out=pa[:], lhsT=lhsT, rhs=acatw_sb[d][:],
                                             start=True, stop=False)
                            nc.tensor.matmul(out=pa[:], lhsT=ones1[:], rhs=acatb_sb[d][0:1, :],
                                             start=False, stop=True)
                            nc.scalar.copy(AXc[:, gj * 192:(gj + 1) * 192], pa[:])

                        # ---- batched tail over the chunk
                        q0 = chkp.tile([P, 2 * G * F], FP32, tag="q0", bufs=1, name="q0")
                        nc.scalar.activation(q0[:, 0:Gc * F], s0c[:, 0:Gc * F], AF.Square)
                        nc.scalar.activation(q0[:, G * F:(G + Gc) * F],
                                             s0c[:, G * F:(G + Gc) * F], AF.Square)

                        scs_r = scs[:].rearrange("p (g s) -> p g s", s=16)

                        def bc(k, w=F):
                            return scs_r[:, 0:Gc, k:k + 1].broadcast_to((P, Gc, w))

                        def r3(tile_ap, w=F):
                            return tile_ap.rearrange("p (g f) -> p g f", f=w)

                        s0lo_v = r3(s0c[:, 0:Gc * F])
                        s0hi_v = r3(s0c[:, G * F:(G + Gc) * F])
                        q0lo_v = r3(q0[:, 0:Gc * F])
                        q0hi_v = r3(q0[:, G * F:(G + Gc) * F])

                        def scr(tag):
                            t = chkp.tile([P, G * F], FP32, tag=tag, name=tag)
                            return t, r3(t[:, 0:Gc * F])

                        t1, t1r = scr("t1")
                        t2, t2r = scr("t2")
                        t3, t3r = scr("t3")
                        t4, t4r = scr("t4")
                        Sb, Sbr = scr("Sb")
                        SQb, SQbr = scr("SQb")
                        MN, MNr = scr("MN")
                        MX, MXr = scr("MX")
                        so = chkp.tile([P, 5 * G * F], FP32, tag="so", bufs=1, name="so")

                        def sov(si):
                            return r3(so[:, si * G * F:(si * G + Gc) * F])

                        mean_o, sfull_o, mn_o, mx_o = sov(0), sov(1), sov(3), sov(4)

                        def stv(s):
                            s = _SLOT[s]
                            return r3(st[:, s * G * F:(s * G + Gc) * F])

                        AXr = r3(AXc[:, 0:Gc * 192], 192)
                        A_v = AXr[:, :, 0:F]
                        X0_v = AXr[:, :, F:192]

                        v.tensor_tensor(t1r, s0lo_v, bc(0), op=OP.mult)
                        v.tensor_tensor(t2r, s0hi_v, bc(1), op=OP.mult)
                        v.tensor_tensor(t3r, t1r, t2r, op=OP.add)
                        v.tensor_tensor(Sbr, stv(0), t3r, op=OP.add)
                        v.tensor_tensor(t1r, q0lo_v, bc(0), op=OP.mult)
                        v.tensor_tensor(t2r, q0hi_v, bc(1), op=OP.mult)
                        v.tensor_tensor(t3r, t1r, t2r, op=OP.add)
                        v.tensor_tensor(SQbr, stv(1), t3r, op=OP.add)
                        is_clean = bool(clean[d][ci])
                        if is_clean:
                            v.tensor_tensor(MNr, stv(3), stv(4), op=OP.min)
                            v.tensor_tensor(MXr, stv(5), stv(6), op=OP.max)
                        else:
                            v.tensor_tensor(t1r, stv(3), bc(2), op=OP.add)
                            v.tensor_tensor(t2r, stv(4), bc(3), op=OP.add)
                            v.tensor_tensor(MNr, t1r, t2r, op=OP.min)
                            v.tensor_tensor(t1r, stv(5), bc(2), op=OP.subtract)
                            v.tensor_tensor(t2r, stv(6), bc(3), op=OP.subtract)
                            v.tensor_tensor(MXr, t1r, t2r, op=OP.max)
                        v.tensor_tensor(t1r, A_v, bc(8), op=OP.mult)
                        v.tensor_tensor(sfull_o, t1r, Sbr, op=OP.add)
                        v.tensor_tensor(mean_o, sfull_o, bc(4), op=OP.mult)
                        v.tensor_tensor(t2r, Sbr, bc(4), op=OP.mult)
                        v.tensor_tensor(t3r, SQbr, bc(4), op=OP.mult)
                        v.tensor_tensor(t4r, t2r, t2r, op=OP.mult)
                        v.tensor_tensor(t1r, t3r, t4r, op=OP.subtract)
                        v.tensor_tensor(t2r, t1r,
                                        zeros[:, 0:1].broadcast_to((P, Gc, F)),
                                        op=OP.max)
                        nc.scalar.activation(so[:, 2 * G * F:(2 * G + Gc) * F],
                                             t2[:, 0:Gc * F], AF.Sqrt,
                                             bias=eps_col[:, 0:1])
                        if is_clean:
                            v.tensor_tensor(mn_o, A_v, MNr, op=OP.add)
                            v.tensor_tensor(mx_o, A_v, MXr, op=OP.add)
                        else:
                            v.tensor_tensor(t3r, A_v, MNr, op=OP.add)
                            v.tensor_tensor(mn_o, t3r, bc(5), op=OP.mult)
                            v.tensor_tensor(t4r, A_v, MXr, op=OP.add)
                            v.tensor_tensor(mx_o, t4r, bc(5), op=OP.mult)

                        if dbg and d == 0 and l == 0:
                            nc.sync.dma_start(out=so_dump[ci, :, :], in_=so[:, 0:5 * G * F])

                        # ---- per-group post matmuls (transposes batched: one
                        # PSUM tile for stats 0-3 + one for stat 4)
                        pysc = pysp.tile([P, G * 192], FP32, tag="pysc", name="pysc")
                        for gj in range(Gc):
                            pt4 = psp.tile([P, 4 * P], FP32, tag="pt4")
                            for pi in range(4):
                                src = so[:, (pi * G + gj) * F:(pi * G + gj + 1) * F]
                                nc.tensor.transpose(out=pt4[:, pi * P:(pi + 1) * P],
                                                    in_=src, identity=ident[:])
                            pt1 = psp.tile([P, P], FP32, tag="ptr")
                            nc.tensor.transpose(out=pt1[:],
                                                in_=so[:, (4 * G + gj) * F:(4 * G + gj + 1) * F],
                                                identity=ident[:])
                            partT4 = xtp.tile([P, 4 * P], FP32, tag="partT4", bufs=1, name="partT4")
                            nc.scalar.copy(partT4[:], pt4[:])
                            partT1 = xtp.tile([P, P], FP32, tag="partT", name="partT")
                            nc.scalar.copy(partT1[:], pt1[:])
                            py = psap.tile([P, 192], FP32, tag="pa")
                            for pi in range(5):
                                lhsT = partT1[:] if pi == 4 else partT4[:, pi * P:(pi + 1) * P]
                                nc.tensor.matmul(out=py[:], lhsT=lhsT,
                                                 rhs=ppw_sb[d][:, pi * 192:(pi + 1) * 192],
                                                 start=(pi == 0), stop=(pi == 4))
                            nc.scalar.copy(pysc[:, gj * 192:(gj + 1) * 192], py[:])
                        pys_r = r3(pysc[:, 0:Gc * 192], 192)
                        yt1r = r3(t1[:, 0:Gc * 64], 64)
                        yt2r = r3(t2[:, 0:Gc * 64], 64)
                        yt3r = r3(t3[:, 0:Gc * 64], 64)
                        y64 = pysp.tile([P, G * 64], FP32, tag="y64", name="y64")
                        y64r = r3(y64[:, 0:Gc * 64], 64)
                        v.tensor_tensor(yt1r, pys_r[:, :, 64:128], bc(6, 64), op=OP.mult)
                        v.tensor_tensor(yt2r, yt1r, pys_r[:, :, 0:64], op=OP.add)
                        v.tensor_tensor(yt1r, pys_r[:, :, 128:192], bc(7, 64), op=OP.mult)
                        v.tensor_tensor(yt3r, yt1r, X0_v, op=OP.add)
                        v.tensor_tensor(y64r, yt2r, yt3r, op=OP.add)

                        # ---- lin is folded into ppw/acatw on the host:
                        # y64 is already h pre-relu
                        hsbc = pysp.tile([P, G * 64], FP32, tag="hsbc", name="hsbc")
                        nc.scalar.activation(hsbc[:, 0:Gc * 64], y64[:, 0:Gc * 64], AF.Relu)

                        if dbg and d == 0 and l == 0:
                            nc.sync.dma_start(out=pys_dump[ci, :, :], in_=pysc[:, 0:G * 192])
                            nc.sync.dma_start(out=hsb_dump[ci, :, :], in_=hsbc[:, 0:G * 64])

                        # ---- scatter previous chunk, defer this one
                        if hsb_prev is not None:
                            scatter_chunk(hsb_prev)
                        hsb_prev = (hsbc, d, g0, Gc)

                if hsb_prev is not None:
                    scatter_chunk(hsb_prev)
                    hsb_prev = None

                if dbg and l == 0:
                    for dd in (0, 1):
                        for hh in range(0, HT, P):
                            hrows = min(P, HT - hh)
                            htt = xtp.tile([P, 64], FP32, tag="htile", name="htile")
                            nc.sync.dma_start(out=htt[:hrows, :], in_=h_tab[dd][hh:hh + hrows, :])
                            nc.sync.dma_start(out=htab_dump[dd][hh:hh + hrows, :], in_=htt[:hrows, :])

                # ---------- phase D: comb (+ next-layer B build when l==0,
                # + head when l==1)
                for g in range(ng):
                    rows = min(P, nown - g * P)
                    if g % G == 0:
                        nb = min(G, ng - g)
                        xpd = xtp.tile([P, nb * P], FP32, tag="xpc",
                                       padded_shape=[P, G * P], name="xpc")
                        src_ap = xTown if l == 0 else x2ownT
                        nc.sync.dma_start(out=xpd[:], in_=src_ap[:, g * P:(g + nb) * P])
                    xlhs_ap = xpd[:, (g % G) * P:((g % G) + 1) * P]
                    hcatT = xtp.tile([P, P], FP32, tag="hcatT", name="hcatT")
                    for d in (0, 1):
                        htile = xtp.tile([P, 64], FP32, tag="htile", name="htile")
                        nc.sync.dma_start(out=htile[:], in_=h_tab[d][g * P:(g + 1) * P, :])
                        pt = psp.tile([P, P], FP32, tag="ptr")
                        nc.tensor.transpose(out=pt[:64, :], in_=htile[:], identity=ident[:])
                        nc.scalar.copy(hcatT[d * 64:(d + 1) * 64, :], pt[:64, :])
                    pc = psp.tile([P, P], FP32, tag="pb")
                    nc.tensor.matmul(out=pc[:], lhsT=xlhs_ap, rhs=combw1_sb[:],
                                     start=True, stop=False)
                    nc.tensor.matmul(out=pc[:], lhsT=hcatT[:], rhs=combw2_sb[:],
                                     start=False, stop=False)
                    nc.tensor.matmul(out=pc[:], lhsT=ones1[:], rhs=combb_sb[0:1, :],
                                     start=False, stop=True)
                    if l < LAYERS - 1:
                        x2n = nlp.tile([P, F], FP32, tag="x2n", name="x2n")
                        nc.scalar.activation(x2n[:], pc[:], AF.Relu)
                        nc.sync.dma_start(out=x2own[g * P:g * P + rows, :],
                                          in_=x2n[:rows, :])
                        # fused layer-1 B build: x2 is already on-chip
                        x2lhsT = transpose_to_sbuf(x2n[:], "x2lT")
                        nc.sync.dma_start(out=x2ownT[:, g * P:(g + 1) * P],
                                          in_=x2lhsT[:])
                        for d in (0, 1):
                            pb = psp.tile([P, P], FP32, tag="pb")
                            nc.tensor.matmul(out=pb[:], lhsT=x2lhsT[:],
                                             rhs=wbot_sb[(1, d)][:],
                                             start=True, stop=True)
                            sb = xtp.tile([P, P], FP32, tag="bsb", name="bsb")
                            nc.scalar.copy(sb[:], pb[:])
                            nc.sync.dma_start(out=Btown[d][g * P:g * P + rows, :],
                                              in_=sb[:rows, :])
                    else:
                        x3 = nlp.tile([P, F], FP32, tag="x3", name="x3")
                        nc.scalar.activation(x3[:], pc[:], AF.Relu)
                        pt = psp.tile([P, P], FP32, tag="ptr")
                        nc.tensor.transpose(out=pt[:], in_=x3[:], identity=ident[:])
                        x3T = xtp.tile([P, P], FP32, tag="x3T", name="x3T")
                        nc.scalar.copy(x3T[:], pt[:])
                        ph = psp.tile([P, 8], FP32, tag="pb")
                        nc.tensor.matmul(out=ph[:], lhsT=x3T[:], rhs=headw_sb[:],
                                         start=True, stop=False)
                        nc.tensor.matmul(out=ph[:], lhsT=ones1[:], rhs=headb_sb[0:1, :],
                                         start=False, stop=True)
                        ot = nlp.tile([P, 8], FP32, tag="ot", name="ot")
                        nc.scalar.copy(ot[:], ph[:])
                        nc.sync.dma_start(out=out_t[g * P:g * P + rows, :],
                                          in_=ot[:rows, :])

    # Spread SWDGE DMAs across the 4 queues, consistently with the DMASW
    # semaphore lane each instruction was assigned (different queues must
    # not increment the same sem — shadow sem tracking).
    if not int(os.environ.get("PNA_NO_QPATCH", "0")):
        from concourse.tile_scheduler import PROC_NAME_TO_IDX
        sw0 = PROC_NAME_TO_IDX["DMASW0"]
        npatched = 0
        for fn in nc.m.functions:
            for bb in fn.blocks:
                for ins in bb.instructions:
                    proc = ins.bass_scheduled_proc
                    if proc is not None and sw0 <= proc < sw0 + 8 and hasattr(ins, "queue_num"):
                        ins.queue_num = (proc - sw0) % NQ
                        npatched += 1
        assert npatched > 0, "no SWDGE DMAs patched - scheduling pass not run?"

    nc.finalize()
    return nc


# ----------------------------------------------------------------- kernel()

def _install_ntff_hook():
    """Register the axon NTFF profile hook if the image's antenv lacks it."""
    import sys
    import types
    try:
        from antenv.axon_hooks import get_axon_ntff_profile_hook  # noqa: F401
        return
    except ImportError:
        pass
    try:
        mod = types.ModuleType("antenv.axon_hooks")
        hook = {"h": None}
        mod.set_axon_ntff_profile_hook = lambda h: hook.__setitem__("h", h)
        mod.get_axon_ntff_profile_hook = lambda: hook["h"]
        sys.modules["antenv.axon_hooks"] = mod
        import antenv
        antenv.axon_hooks = mod
        from trn_agent_boot.trn_boot import _ntff_profile_via_ctypes
        mod.set_axon_ntff_profile_hook(
            _ntff_profile_via_ctypes("/opt/axon/libaxon_pjrt.so"))
    except Exception:
        pass


def _prep_all(inputs):
    x = np.asarray(inputs["x"], np.float32)
    n_nodes, f = x.shape
    assert f == F
    assert n_nodes % NCORES == 0
    nown = n_nodes // NCORES
    half = ((n_nodes // 2 + 1 + P - 1) // P) * P
    assert half < 32768 and 2 * half > n_nodes
    tbl = 2 * half

    avg_in = float(np.asarray(inputs["avg_in"]))
    avg_out = float(np.asarray(inputs["avg_out"]))
    low_w = min(32768, tbl)
    hib = max(0, tbl - 32768)
    assert n_nodes - hib < 32768

    prep = {}
    Ksched = {}
    for d, (ei, avg) in enumerate(
            ((inputs["edge_index_in"], avg_in), (inputs["edge_index_out"], avg_out))):
        cores, K_lo, K_hi, ng, nownp, chunks, g0s, clean = _prep_direction(
            ei, n_nodes, nown, low_w, hib, avg)
        prep[d] = cores
        Ksched[d] = (K_lo, K_hi, clean)

    meta = dict(n_nodes=n_nodes, low_w=low_w, hib=hib, tbl=tbl, nown=nown,
                nownp=nownp, ng=ng, chunks=chunks, g0s=g0s,
                K_lo={d: Ksched[d][0] for d in (0, 1)},
                K_hi={d: Ksched[d][1] for d in (0, 1)},
                clean={d: Ksched[d][2] for d in (0, 1)})

    wl = [_prep_weights(inputs, l) for l in range(LAYERS)]
    shared = dict(
        wbot=np.stack([np.stack([wl[l][(d, "wbot")] for d in (0, 1)]) for l in range(LAYERS)]),
        acatw=np.stack([np.stack([wl[l][(d, "acatw")] for d in (0, 1)]) for l in range(LAYERS)]),
        acatb=np.stack([np.stack([wl[l][(d, "acatb")] for d in (0, 1)]) for l in range(LAYERS)]),
        ppw=np.stack([np.stack([wl[l][(d, "ppw")] for d in (0, 1)]) for l in range(LAYERS)]),
        combw=np.stack([wl[l]["combw"] for l in range(LAYERS)]),
        combb=np.stack([wl[l]["combb"] for l in range(LAYERS)]),
        headw=np.asarray(inputs["head_W"], np.float32),
        headb=np.tile(np.asarray(inputs["head_b"], np.float32)[None, :], (P, 1)),
    )

    in_maps = []
    for c in range(NCORES):
        nownp = meta["nownp"]
        xTown_np = np.zeros((P, nownp), np.float32)
        xown = x[c * nown:(c + 1) * nown]
        xTown_np[:, :nown] = xown.T
        m = dict(xTown=xTown_np, **shared)
        for d in (0, 1):
            pc = prep[d][c]
            pp = pc["perm_pad"]
            xP = np.zeros((P, nownp), np.float32)
            xP[:, :nown] = xown[pp[:nown]].T
            m[f"xTownP{d}"] = xP
            m[f"idxcat{d}"] = pc["idx_cat"]
            m[f"scal{d}"] = pc["scal"]
            m[f"x2pidx{d}"] = pc["x2pidx"]
            m[f"hidx{d}"] = pc["h_idx"]
        in_maps.append(m)
    return meta, in_maps


def kernel(**inputs):
    meta, in_maps = _prep_all(inputs)
    nc = build_program(meta)
    trace = bool(int(os.environ.get("PNA_TRACE", "0")))
    if trace:
        _install_ntff_hook()
    res = run_bass_kernel_spmd(nc, in_maps, core_ids=list(range(NCORES)),
                               trace=trace)
    if trace and res.exec_time_ns is not None:
        print(f"HW exec time: {res.exec_time_ns} ns")
    out = np.concatenate([res.results[c]["out"] for c in range(NCORES)], axis=0)
    return out.astype(np.float32)
